# revision 1
# baseline (speedup 1.0000x reference)
"""Causal self-attention + residual + LayerNorm fused Trainium2 kernel.

Problem: B=4, S=2048, D=1024, H=16 heads (hd=64), fp32 in/out.
    qkv = x @ in_proj_w.T + in_proj_b ; causal MHA ; out proj ; y = LN(x + attn_out)

Sharding (zero cross-core communication, 8 NeuronCores):
    core c -> batch b = c % 4, query-group g = c // 4.
    Causal zig-zag balance: g=0 owns query blocks [0:512) and [1536:2048),
    g=1 owns [512:1536). Every core computes full K/V for its batch
    (keys 0:2048), attention only for its own queries, then out-proj +
    residual + LayerNorm for its queries. Outputs are disjoint row sets.

Layout: everything is computed transposed (features on partitions,
tokens on the free axis), which makes every matmul contraction land on
the partition axis with zero on-chip transposes:
    K^T[f,t] / Q^T[f,q] = W^T-tile.T @ x^T        (lhsT = in_proj_w.T tile)
    V[t,f]              = x^T-tile.T @ W^T        (lhsT = x^T tile)
    S^T[k,q]            = K^T-slice.T @ Q^T-slice (contraction = head dim 64,
                                                   two heads packed in the PE
                                                   array via tile_position)
    ctx^T[d,q]          = V-slice.T @ exp(S^T)    (V augmented with a ones
                                                   column -> row 64 of the
                                                   PSUM tile = softmax denom)
    out^T[Do,q]         = out_w.T-tile.T @ ctx^T
    LN stats            = ones.T @ y / ones.T @ y^2 (partition reduction on PE)
Matmuls run in float32r (TF32-like, ~11-bit mantissa, 4x faster than fp32
on the PE; measured end-to-end relerr ~1.5e-4). Softmax skips the max
subtraction (scores ~ N(0,1)) and defers the divide: ctx is normalized by
the reciprocal of the aug-row denominator, broadcast across partitions by
the GPSIMD partition_broadcast op.

The two query-groups differ only in the per-q-tile causal k-tile counts;
both variants are emitted under a tc.If on the partition id, so one SPMD
program serves all 8 cores in a single launch.
"""
import sys

if "/opt/trn_rl_repo" not in sys.path:
    sys.path.insert(0, "/opt/trn_rl_repo")

import numpy as np

B, S, D, H, HD = 4, 2048, 1024, 16, 64
P = 128
QT = 512                      # queries per q-tile (matmul free dim)
NQ = 1024                     # queries per core
NKT = S // P                  # 16 k-tiles per batch
DK = D // P                   # 8 contraction tiles over D
NPLAIN = {0: (0, 12), 1: (4, 8)}   # group -> per-q-tile plain (unmasked) k-tiles

_cache = {}


def _build():
    import concourse.mybir as mybir
    import concourse.tile as tile
    from concourse import bacc
    from concourse.bass import ts
    from concourse.alu_op_type import AluOpType

    f32 = mybir.dt.float32
    f32r = mybir.dt.float32r
    AF = mybir.ActivationFunctionType

    nc = bacc.Bacc("TRN2", target_bir_lowering=False, debug=False, num_devices=8)

    xkv = nc.dram_tensor("xkv", [D, S], f32r, kind="ExternalInput").ap()
    xq = nc.dram_tensor("xq", [D, NQ], f32r, kind="ExternalInput").ap()
    wt = nc.dram_tensor("wt", [D, 3 * D], f32r, kind="ExternalInput").ap()
    wot = nc.dram_tensor("wot", [D, D], f32r, kind="ExternalInput").ap()
    maskd = nc.dram_tensor("maskd", [P, 896], f32r, kind="ExternalInput").ap()
    bqd = nc.dram_tensor("bqd", [D], f32, kind="ExternalInput").ap()
    bkd = nc.dram_tensor("bkd", [D], f32, kind="ExternalInput").ap()
    bvd = nc.dram_tensor("bvd", [D], f32, kind="ExternalInput").ap()
    bod = nc.dram_tensor("bod", [D], f32, kind="ExternalInput").ap()
    gamd = nc.dram_tensor("gamd", [D], f32, kind="ExternalInput").ap()
    betd = nc.dram_tensor("betd", [D], f32, kind="ExternalInput").ap()
    yt = nc.dram_tensor("yt", [D, NQ], f32, kind="ExternalOutput").ap()

    xkv_r = xkv.rearrange("(dk p) t -> p dk t", p=P)
    xq_r = xq.rearrange("(dk p) q -> p dk q", p=P)
    xq_f32 = xq.bitcast(f32).rearrange("(ok p) q -> p ok q", p=P)

    with tile.TileContext(nc) as tc:
        with (
            tc.tile_pool(name="persist", bufs=1) as pers,
            tc.tile_pool(name="proj_ps", bufs=2, space="PSUM") as proj_ps,
        ):
            kt = pers.tile([P, DK, S], f32r)           # K^T       64 KB/part
            msk = pers.tile([P, 896], f32r)            #           3.5 KB
            bia = pers.tile([P, DK, 6], f32)           # bq bk bv bo gam bet
            ones128 = pers.tile([P, 1], f32r)
            eps_t = pers.tile([1, 1], f32)
            nc.vector.memset(eps_t[:], 1e-5)

            nc.sync.dma_start(msk[:], maskd[:])
            for j, src in enumerate((bqd, bkd, bvd, bod, gamd, betd)):
                nc.sync.dma_start(bia[:, :, j], src.rearrange("(f p) -> p f", p=P))
            nc.vector.memset(ones128[:].bitcast(f32), 1.0)

            def bq_(f): return bia[:, f, 0:1]
            def bk_(f): return bia[:, f, 1:2]
            def bo_(f): return bia[:, f, 3:4]
            def gam_(f): return bia[:, f, 4:5]
            def bet_(f): return bia[:, f, 5:6]

            # ---- phase A: K^T projection ------------------------------
            with (
                tc.tile_pool(name="wk", bufs=1) as wkp,
                tc.tile_pool(name="xa", bufs=2) as xap,
            ):
                wk = wkp.tile([P, DK, DK, P], f32r)
                nc.sync.dma_start(
                    wk[:],
                    wt[:, D:2 * D].rearrange("(dk p) (f c) -> p dk f c", p=P, c=P),
                )
                for t in range(S // QT):
                    xc = xap.tile([P, DK, QT], f32r, tag="xa")
                    nc.sync.dma_start(xc[:], xkv_r[:, :, ts(t, QT)])
                    for f in range(DK):
                        ps = proj_ps.tile([P, QT], f32, tag="pp")
                        for dk in range(DK):
                            nc.tensor.matmul(
                                ps[:], wk[:, dk, f, :], xc[:, dk, :],
                                start=(dk == 0), stop=(dk == DK - 1),
                            )
                        nc.vector.tensor_scalar_add(kt[:, f, ts(t, QT)], ps[:], bk_(f))

            with tc.tile_pool(name="vpool", bufs=1) as vp:
                v = vp.tile([P, NKT, H, HD + 1], f32r)   # V aug  65 KB/part
                nc.vector.memset(v[:, :, :, HD].bitcast(f32), 1.0)

                # ---- phase B: V projection (natural orientation) ------
                with (
                    tc.tile_pool(name="wv", bufs=1) as wvp,
                    tc.tile_pool(name="xb", bufs=4) as xbp,
                ):
                    wv = wvp.tile([P, DK, 2, 512], f32r)
                    nc.sync.dma_start(
                        wv[:],
                        wt[:, 2 * D:3 * D].rearrange(
                            "(dk p) (g c) -> p dk g c", p=P, c=512),
                    )
                    for t in range(NKT):
                        xc = xbp.tile([P, DK, P], f32r, tag="xb")
                        nc.sync.dma_start(xc[:], xkv_r[:, :, ts(t, P)])
                        for fg in range(2):
                            ps = proj_ps.tile([P, 512], f32, tag="pp")
                            for dk in range(DK):
                                nc.tensor.matmul(
                                    ps[:], xc[:, dk, :], wv[:, dk, fg, :],
                                    start=(dk == 0), stop=(dk == DK - 1),
                                )
                            for hh in range(8):
                                h = 8 * fg + hh
                                nc.vector.tensor_copy(
                                    v[:, t, h, 0:HD], ps[:, ts(hh, HD)]
                                )

                # ---- phases C-F under the partition-id branch ---------
                with tc.tile_pool(name="qc", bufs=1) as qcp:
                    ctx = qcp.tile([P, DK, QT], f32r)

                    def qproj(qt, qtile):
                        with (
                            tc.tile_pool(name="wq", bufs=2) as wqp,
                            tc.tile_pool(name="xqp", bufs=1) as xqp,
                        ):
                            xc = xqp.tile([P, DK, QT], f32r, tag="xq")
                            nc.sync.dma_start(xc[:], xq_r[:, :, ts(qt, QT)])
                            for f in range(DK):
                                wq = wqp.tile([P, DK, P], f32r, tag="wq")
                                nc.sync.dma_start(
                                    wq[:],
                                    wt[:, ts(f, P)].rearrange(
                                        "(dk p) c -> p dk c", p=P),
                                )
                                ps = proj_ps.tile([P, QT], f32, tag="pp")
                                for dk in range(DK):
                                    nc.tensor.matmul(
                                        ps[:], wq[:, dk, :], xc[:, dk, :],
                                        start=(dk == 0), stop=(dk == DK - 1),
                                    )
                                nc.vector.tensor_scalar_add(
                                    qtile[:, f, :], ps[:], bq_(f))

                    def attn(n_plain, qtile):
                        nk = n_plain + 4
                        with (
                            tc.tile_pool(name="sep", bufs=4) as sep,
                            tc.tile_pool(name="scr", bufs=2) as scr,
                            tc.tile_pool(name="s_ps", bufs=2, space="PSUM") as s_ps,
                            tc.tile_pool(name="c_ps", bufs=2, space="PSUM") as c_ps,
                        ):
                            for hp in range(H // 2):
                                cp0 = c_ps.tile([HD + 1, QT], f32, tag="c0")
                                cp1 = c_ps.tile([HD + 1, QT], f32, tag="c1")
                                for i in range(nk):
                                    sp0 = s_ps.tile([P, QT], f32, tag="s")
                                    sp1 = s_ps.tile([P, QT], f32, tag="s")
                                    nc.tensor.matmul(
                                        sp0[:], kt[0:HD, hp, ts(i, P)],
                                        qtile[0:HD, hp, :], start=True, stop=True,
                                    )
                                    nc.tensor.matmul(
                                        sp1[:], kt[HD:P, hp, ts(i, P)],
                                        qtile[HD:P, hp, :], start=True, stop=True,
                                    )
                                    se0 = sep.tile([P, QT], f32r, tag="se")
                                    se1 = sep.tile([P, QT], f32r, tag="se")
                                    nc.scalar.activation(
                                        se0[:], sp0[:], AF.Exp, scale=0.125)
                                    nc.scalar.activation(
                                        se1[:], sp1[:], AF.Exp, scale=0.125)
                                    if i >= n_plain:
                                        off = 384 - P * (i - n_plain)
                                        nc.vector.tensor_mul(
                                            se0[:], se0[:], msk[:, off:off + QT])
                                        nc.vector.tensor_mul(
                                            se1[:], se1[:], msk[:, off:off + QT])
                                    nc.tensor.matmul(
                                        cp0[:], v[:, i, 2 * hp, :], se0[:],
                                        start=(i == 0), stop=(i == nk - 1),
                                    )
                                    nc.tensor.matmul(
                                        cp1[:], v[:, i, 2 * hp + 1, :], se1[:],
                                        start=(i == 0), stop=(i == nk - 1),
                                    )
                                for j, cp in ((0, cp0), (1, cp1)):
                                    h = 2 * hp + j
                                    po, ft = HD * (h % 2), h // 2
                                    den = scr.tile([1, QT], f32, tag="den")
                                    nc.vector.tensor_copy(den[:], cp[HD:HD + 1, :])
                                    rec = scr.tile([1, QT], f32, tag="rec")
                                    rscr = scr.tile([1, QT], f32, tag="rscr")
                                    nc.vector.reciprocal_approx_accurate(
                                        rec[:], den[:], rscr[:])
                                    bc = scr.tile([HD, QT], f32, tag="bc")
                                    nc.gpsimd.partition_broadcast(bc[:], rec[:])
                                    dst = ctx[po:po + HD, ft, :]
                                    nc.vector.tensor_mul(dst, cp[0:HD, :], bc[:])
                                    nc.vector.tensor_scalar_add(
                                        dst, dst, bia[po:po + HD, ft, 2:3])

                    def outproj_ln(qt):
                        with (
                            tc.tile_pool(name="wo", bufs=3) as wop,
                            tc.tile_pool(name="ep", bufs=1) as ep,
                            tc.tile_pool(name="st_ps", bufs=2, space="PSUM") as st_ps,
                        ):
                            y = ep.tile([P, DK, QT], f32r, tag="y")
                            for o in range(DK):
                                wo = wop.tile([P, DK, P], f32r, tag="wo")
                                nc.sync.dma_start(
                                    wo[:],
                                    wot[:, ts(o, P)].rearrange(
                                        "(dk p) c -> p dk c", p=P),
                                )
                                ps = proj_ps.tile([P, QT], f32, tag="pp")
                                for dk in range(DK):
                                    nc.tensor.matmul(
                                        ps[:], wo[:, dk, :], ctx[:, dk, :],
                                        start=(dk == 0), stop=(dk == DK - 1),
                                    )
                                xr = ep.tile([P, QT], f32, tag="xr", bufs=3)
                                nc.sync.dma_start(xr[:], xq_f32[:, o, ts(qt, QT)])
                                nc.vector.scalar_tensor_tensor(
                                    y[:, o, :], ps[:], bo_(o), xr[:],
                                    AluOpType.add, AluOpType.add,
                                )
                            mu_ps = st_ps.tile([1, QT], f32, tag="mu")
                            for o in range(DK):
                                nc.tensor.matmul(
                                    mu_ps[:], ones128[:], y[:, o, :],
                                    start=(o == 0), stop=(o == DK - 1))
                            ms_ps = st_ps.tile([1, QT], f32, tag="ms")
                            for o in range(DK):
                                ysq = ep.tile([P, QT], f32r, tag="ysq")
                                nc.vector.tensor_mul(
                                    ysq[:], y[:, o, :], y[:, o, :])
                                nc.tensor.matmul(
                                    ms_ps[:], ones128[:], ysq[:],
                                    start=(o == 0), stop=(o == DK - 1))
                            mu = ep.tile([1, QT], f32, tag="mu_sb")
                            nc.scalar.mul(mu[:], mu_ps[:], 1.0 / D)
                            ms = ep.tile([1, QT], f32, tag="ms_sb")
                            nc.scalar.mul(ms[:], ms_ps[:], 1.0 / D)
                            tmp = ep.tile([1, QT], f32, tag="stat_tmp", bufs=2)
                            nc.vector.tensor_mul(tmp[:], mu[:], mu[:])
                            nc.vector.tensor_sub(ms[:], ms[:], tmp[:])  # var
                            sd = ep.tile([1, QT], f32, tag="stat_tmp", bufs=2)
                            nc.scalar.activation(sd[:], ms[:], AF.Sqrt, bias=eps_t[:])
                            rstd = ep.tile([1, QT], f32, tag="rstd")
                            rsc = ep.tile([1, QT], f32, tag="stat_tmp", bufs=2)
                            nc.vector.reciprocal_approx_accurate(
                                rstd[:], sd[:], rsc[:])
                            mu_bc = ep.tile([P, QT], f32, tag="mu_bc")
                            nc.gpsimd.partition_broadcast(mu_bc[:], mu[:])
                            rs_bc = ep.tile([P, QT], f32, tag="rs_bc")
                            nc.gpsimd.partition_broadcast(rs_bc[:], rstd[:])
                            for o in range(DK):
                                t1 = ep.tile([P, QT], f32, tag="t1", bufs=2)
                                nc.vector.tensor_sub(
                                    t1[:], y[:, o, :].bitcast(f32), mu_bc[:])
                                nc.vector.tensor_mul(t1[:], t1[:], rs_bc[:])
                                yo = ep.tile([P, QT], f32, tag="yo", bufs=2)
                                nc.vector.tensor_scalar(
                                    yo[:], t1[:], gam_(o), bet_(o),
                                    AluOpType.mult, AluOpType.add,
                                )
                                nc.sync.dma_start(yt[ts(o, P), ts(qt, QT)], yo[:])

                    def group(g):
                        for qt in range(2):
                            with tc.tile_pool(name="qtp", bufs=1) as qtp:
                                qtile = qtp.tile([P, DK, QT], f32r, tag="qtile")
                                qproj(qt, qtile)
                                attn(NPLAIN[g][qt], qtile)
                            outproj_ln(qt)

                    pid = nc.partition_id()
                    with tc.If(pid < 4) as cmp:
                        group(0)
                    with cmp.Else():
                        group(1)
    nc.compile()
    return nc


def _get_nc():
    if "nc" not in _cache:
        _cache["nc"] = _build()
    return _cache["nc"]


def _prep(x, in_proj_w, in_proj_b, out_w, out_b, gamma, beta):
    x = np.asarray(x, np.float32)
    wt = np.ascontiguousarray(np.asarray(in_proj_w, np.float32).T)
    wot = np.ascontiguousarray(np.asarray(out_w, np.float32).T)
    bqkv = np.asarray(in_proj_b, np.float32)
    bo = np.asarray(out_b, np.float32)
    gam = np.asarray(gamma, np.float32)
    bet = np.asarray(beta, np.float32)
    ku = np.arange(P)[:, None] <= (np.arange(896)[None, :] - 384)
    maskd = ku.astype(np.float32)
    qcols = {
        0: np.r_[0:QT, 3 * QT:4 * QT],
        1: np.r_[QT:3 * QT],
    }
    in_maps = []
    for c in range(8):
        b, g = c % 4, c // 4
        xt = np.ascontiguousarray(x[b].T)
        in_maps.append({
            "xkv": xt,
            "xq": np.ascontiguousarray(xt[:, qcols[g]]),
            "wt": wt,
            "wot": wot,
            "maskd": maskd,
            "bqd": bqkv[0:D], "bkd": bqkv[D:2 * D], "bvd": bqkv[2 * D:3 * D],
            "bod": bo, "gamd": gam, "betd": bet,
        })
    return in_maps, qcols


def _run(in_maps, trace=False, **kw):
    from concourse.bass_utils import run_bass_kernel_spmd

    return run_bass_kernel_spmd(_get_nc(), in_maps, list(range(8)), trace=trace, **kw)


def kernel(x, in_proj_w, in_proj_b, out_w, out_b, gamma, beta):
    in_maps, qcols = _prep(x, in_proj_w, in_proj_b, out_w, out_b, gamma, beta)
    res = _run(in_maps)
    out = np.empty((B, S, D), np.float32)
    for c in range(8):
        out[c % 4, qcols[c // 4]] = res.results[c]["yt"].T
    return out



# revision 5
# speedup vs baseline: 1.6015x; 1.6015x over previous
"""Causal self-attention + residual + LayerNorm fused Trainium2 kernel (v2).

Problem: B=4, S=2048, D=1024, H=16 heads (hd=64), fp32 in/out.
    qkv = x @ in_proj_w.T + in_proj_b ; causal MHA ; out proj ; y = LN(x + attn_out)

Sharding (zero cross-core communication, 8 NeuronCores):
    core c -> batch b = c % 4, query-group g = c // 4.
    Causal zig-zag balance: g=0 owns query blocks [0:512) and [1536:2048),
    g=1 owns [512:1536). Every core computes full K/V for its batch,
    attention only for its own 1024 queries, then out-proj + residual +
    LayerNorm for its queries. Outputs are disjoint row sets.

v2 changes vs v1 (820us):
  - All matmul operands bf16 (same PE rate as f32r at free>=256, but FWL
    halves LDWEIGHTS and DMA bytes halve; DVE gets 2x on 16-bit ops).
  - Projections restructured so one LDWEIGHTS feeds 2-4 matmuls.
  - Whole per-core program lives inside one tc.If branch so the Tile
    scheduler can interleave projections / attention / LN freely -> PE
    stays dense enough to keep the HAM clock gate at 2.4 GHz (v1 spent
    546us of 821us throttled at 1.2 GHz).
  - Causal diagonal band (last 4 k-tiles of each q-tile) restricts the
    matmul/exp/mask free dim to the valid query suffix; mask multiply
    shrinks to one [128,2x128] bf16 op on the diagonal block only.
  - exp for both heads of a pair merged into one [128,2,F] ACTIVATE from
    a 2-bank PSUM tile.
  - softmax denominator: V is augmented with a ones column (PSUM row 64),
    reciprocal_approx_fast on [1,2,512] per head-pair, partition-broadcast
    on GpSimd, normalize on DVE.
  - LayerNorm rstd via exp(-0.5*ln(var+eps)) -- keeps the single Act
    table (exp/identity/copy/square/ln) loaded, no table swaps.
  - V projection bias folded into the out-proj bias on the host
    (softmax rows sum to 1): bo' = out_b + out_w @ bv.
"""
import sys

if "/opt/trn_rl_repo" not in sys.path:
    sys.path.insert(0, "/opt/trn_rl_repo")

import numpy as np

B, S, D, H, HD = 4, 2048, 1024, 16, 64
P = 128
QT = 512                      # queries per q-tile
NQ = 1024                     # queries per core
NKT = S // P                  # 16 k-tiles per batch
DK = D // P                   # 8 contraction tiles over D
QSTART = {0: (0, 3 * QT), 1: (QT, 2 * QT)}   # group -> per-q-tile query start
NKS = {0: (4, 16), 1: (8, 12)}               # group -> per-q-tile k-tile count

_cache = {}


def _build():
    import concourse.mybir as mybir
    import concourse.tile as tile
    from concourse import bacc
    from concourse.bass import ts
    from concourse.alu_op_type import AluOpType

    f32 = mybir.dt.float32
    f32r = mybir.dt.float32r
    bf16 = mybir.dt.bfloat16
    AF = mybir.ActivationFunctionType

    nc = bacc.Bacc("TRN2", target_bir_lowering=False, debug=False, num_devices=8)

    xq32 = nc.dram_tensor("xq32", [D, NQ], f32, kind="ExternalInput").ap()
    xtb = nc.dram_tensor("xtb", [D, S], bf16, kind="ExternalInput").ap()
    wqkv = nc.dram_tensor("wqkv", [D, 3 * D], bf16, kind="ExternalInput").ap()
    wotd = nc.dram_tensor("wotd", [D, D], bf16, kind="ExternalInput").ap()
    mskd = nc.dram_tensor("mskd", [P, 2 * P], bf16, kind="ExternalInput").ap()
    bqd = nc.dram_tensor("bqd", [D], f32, kind="ExternalInput").ap()
    bkd = nc.dram_tensor("bkd", [D], f32, kind="ExternalInput").ap()
    bod = nc.dram_tensor("bod", [D], f32, kind="ExternalInput").ap()
    gamd = nc.dram_tensor("gamd", [D], f32, kind="ExternalInput").ap()
    betd = nc.dram_tensor("betd", [D], f32, kind="ExternalInput").ap()
    yt = nc.dram_tensor("yt", [D, NQ], f32, kind="ExternalOutput").ap()

    xt_r = xtb.rearrange("(dk p) t -> p dk t", p=P)
    xq32_r = xq32.rearrange("(ok p) q -> p ok q", p=P)
    w_r = wqkv.rearrange("(dk p) (s c) -> p dk s c", p=P, c=P)  # 24 sections
    wot_r = wotd.rearrange("(dk p) (o c) -> p dk o c", p=P, c=P)

    with tile.TileContext(nc) as tc:
        with tc.tile_pool(name="pers", bufs=1) as pers:
            msk = pers.tile([P, 2, P], bf16)
            bias = pers.tile([P, DK, 5], f32)      # bq bk bo' gam bet
            ones = pers.tile([P, 1], f32r)
            eps = pers.tile([1, 1], f32)
            nc.vector.memset(eps[:], 1e-5)
            nc.vector.memset(ones[:].bitcast(f32), 1.0)
            nc.sync.dma_start(msk[:], mskd.rearrange("p (h c) -> p h c", c=P))
            for j, src in enumerate((bqd, bkd, bod, gamd, betd)):
                nc.sync.dma_start(bias[:, :, j], src.rearrange("(f p) -> p f", p=P))

            def bq_(f): return bias[:, f, 0:1]
            def bk_(f): return bias[:, f, 1:2]
            def bo_(f): return bias[:, f, 2:3]
            def gam_(f): return bias[:, f, 3:4]
            def bet_(f): return bias[:, f, 4:5]

            def body(g):
                qss, nks = QSTART[g], NKS[g]
                with tc.tile_pool(name="big", bufs=1) as big:
                    kt = big.tile([P, DK, S], bf16, name=f"kt{g}")
                    v = big.tile([P, NKT, H, HD + 1], bf16, name=f"v{g}")
                    qa = big.tile([P, DK, NQ], bf16, name=f"qa{g}")
                    ctxt = big.tile([P, DK, QT], bf16, name=f"ctxt{g}")
                    y = big.tile([P, DK, QT], f32r, name=f"y{g}")
                    nc.vector.memset(v[:, :, :, HD], 1.0)

                    # ---- projections: K, V, Q -------------------------
                    with (
                        tc.tile_pool(name="xtp", bufs=1) as xtp,
                        tc.tile_pool(name="wsp", bufs=2) as wsp,
                        tc.tile_pool(name="pp8", bufs=4, space="PSUM") as pp8,
                    ):
                        xt = xtp.tile([P, DK, S], bf16, name=f"xt{g}")
                        for t in range(4):
                            nc.sync.dma_start(
                                xt[:, :, ts(t, QT)], xt_r[:, :, ts(t, QT)])

                        # K^T: stationary w chunk reused over 4 t-tiles
                        wk = wsp.tile([P, DK, 8, P], bf16, tag="w", name=f"wk{g}")
                        nc.sync.dma_start(wk[:], w_r[:, :, 8:16, :])
                        for f in range(DK):
                            pts = [
                                pp8.tile([P, QT], f32, tag="pp", name=f"pk{g}_{f}_{t}")
                                for t in range(4)
                            ]
                            for dk in range(DK):
                                for t in range(4):
                                    nc.tensor.matmul(
                                        pts[t][:], wk[:, dk, f, :],
                                        xt[:, dk, ts(t, QT)],
                                        start=(dk == 0), stop=(dk == DK - 1),
                                    )
                            for t in range(4):
                                nc.scalar.add(kt[:, f, ts(t, QT)], pts[t][:], bk_(f))

                        # V natural: stationary x chunk reused over 2 f-groups
                        wv = wsp.tile([P, DK, 8, P], bf16, tag="w", name=f"wv{g}")
                        nc.sync.dma_start(wv[:], w_r[:, :, 16:24, :])
                        for t in range(NKT):
                            pv = [
                                pp8.tile([P, 8, HD], f32, tag="pp", name=f"pv{g}_{t}_{fg}")
                                for fg in range(2)
                            ]
                            for dk in range(DK):
                                for fg in range(2):
                                    nc.tensor.matmul(
                                        pv[fg][:], xt[:, dk, ts(t, P)],
                                        wv[:, dk, 4 * fg:4 * (fg + 1), :],
                                        start=(dk == 0), stop=(dk == DK - 1),
                                    )
                            for fg in range(2):
                                nc.scalar.copy(
                                    v[:, t, 8 * fg:8 * (fg + 1), 0:HD], pv[fg][:])

                        # Q^T: stationary w chunk reused over both q-tiles
                        wq = wsp.tile([P, DK, 8, P], bf16, tag="w", name=f"wq{g}")
                        nc.sync.dma_start(wq[:], w_r[:, :, 0:8, :])
                        for f in range(DK):
                            pq = [
                                pp8.tile([P, QT], f32, tag="pp", name=f"pq{g}_{f}_{qt}")
                                for qt in range(2)
                            ]
                            for dk in range(DK):
                                for qt in range(2):
                                    nc.tensor.matmul(
                                        pq[qt][:], wq[:, dk, f, :],
                                        xt[:, dk, qss[qt]:qss[qt] + QT],
                                        start=(dk == 0), stop=(dk == DK - 1),
                                    )
                            for qt in range(2):
                                nc.scalar.add(qa[:, f, ts(qt, QT)], pq[qt][:], bq_(f))

                    # ---- attention + out-proj + LN --------------------
                    with (
                        tc.tile_pool(name="sps", bufs=2, space="PSUM") as sps,
                        tc.tile_pool(name="cps", bufs=1, space="PSUM") as cps,
                        tc.tile_pool(name="pp2", bufs=2, space="PSUM") as pp2,
                        tc.tile_pool(name="sep", bufs=3) as sep,
                        tc.tile_pool(name="wop", bufs=2) as wop,
                        tc.tile_pool(name="ep", bufs=2) as ep,
                    ):
                        for qt in range(2):
                            nk = nks[qt]
                            qb = qt * QT
                            # ---- attention ----
                            for hp in range(H // 2):
                                cp = cps.tile([P, 2, QT], f32, tag="cp",
                                              name=f"cp{g}_{qt}_{hp}")
                                for i in range(nk):
                                    bi = i - (nk - 4)
                                    off = P * bi if bi > 0 else 0
                                    sp = sps.tile([P, 2, QT], f32, tag="sp",
                                                  name=f"sp{g}_{qt}_{hp}_{i}")
                                    nc.tensor.matmul(
                                        sp[:, 0, off:], kt[0:HD, hp, ts(i, P)],
                                        qa[0:HD, hp, qb + off:qb + QT],
                                        start=True, stop=True,
                                    )
                                    nc.tensor.matmul(
                                        sp[:, 1, off:], kt[HD:P, hp, ts(i, P)],
                                        qa[HD:P, hp, qb + off:qb + QT],
                                        start=True, stop=True,
                                    )
                                    se = sep.tile([P, 2, QT], bf16, tag="se",
                                                  name=f"se{g}_{qt}_{hp}_{i}")
                                    nc.scalar.activation(
                                        se[:, :, off:], sp[:, :, off:],
                                        AF.Exp, scale=0.125)
                                    if bi >= 0:
                                        nc.vector.tensor_mul(
                                            se[:, :, off:off + P],
                                            se[:, :, off:off + P], msk[:])
                                    nc.tensor.matmul(
                                        cp[0:HD + 1, 0, off:], v[:, i, 2 * hp, :],
                                        se[:, 0, off:],
                                        start=(i == 0), stop=(i == nk - 1),
                                    )
                                    nc.tensor.matmul(
                                        cp[0:HD + 1, 1, off:], v[:, i, 2 * hp + 1, :],
                                        se[:, 1, off:],
                                        start=(i == 0), stop=(i == nk - 1),
                                    )
                                den = ep.tile([1, 2, QT], f32, tag="den",
                                              name=f"den{g}_{qt}_{hp}")
                                nc.vector.tensor_copy(den[:], cp[HD:HD + 1, :, :])
                                for j in range(2):
                                    rden = ep.tile([1, QT], f32, tag="rden",
                                                   name=f"rden{g}_{qt}_{hp}_{j}")
                                    nc.vector.reciprocal_approx_fast(
                                        rden[:], den[:, j, :])
                                    bch = ep.tile([HD, QT], f32, tag="bch",
                                                  name=f"bch{g}_{qt}_{hp}_{j}")
                                    nc.gpsimd.partition_broadcast(bch[:], rden[:])
                                    nc.vector.tensor_mul(
                                        ctxt[j * HD:(j + 1) * HD, hp, :],
                                        cp[0:HD, j, :], bch[:])

                            # ---- out proj + residual + LN ----
                            for o in range(DK):
                                wo = wop.tile([P, DK, P], bf16, tag="wo",
                                              name=f"wo{g}_{qt}_{o}")
                                nc.sync.dma_start(wo[:], wot_r[:, :, o, :])
                                ps = pp2.tile([P, QT], f32, tag="pp2",
                                              name=f"po{g}_{qt}_{o}")
                                for dk in range(DK):
                                    nc.tensor.matmul(
                                        ps[:], wo[:, dk, :], ctxt[:, dk, :],
                                        start=(dk == 0), stop=(dk == DK - 1),
                                    )
                                xr = ep.tile([P, QT], f32, tag="xr", bufs=3,
                                             name=f"xr{g}_{qt}_{o}")
                                nc.sync.dma_start(xr[:], xq32_r[:, o, ts(qt, QT)])
                                nc.vector.scalar_tensor_tensor(
                                    y[:, o, :], ps[:], bo_(o), xr[:],
                                    AluOpType.add, AluOpType.add,
                                )
                            mu_ps = pp2.tile([P, QT], f32, tag="pp2",
                                             name=f"mups{g}_{qt}")
                            for o in range(DK):
                                nc.tensor.matmul(
                                    mu_ps[0:1, :], ones[:], y[:, o, :],
                                    start=(o == 0), stop=(o == DK - 1))
                            ms_ps = pp2.tile([P, QT], f32, tag="pp2",
                                             name=f"msps{g}_{qt}")
                            for o in range(DK):
                                ysq = ep.tile([P, QT], f32r, tag="ysq",
                                              name=f"ysq{g}_{qt}_{o}")
                                nc.vector.tensor_mul(
                                    ysq[:], y[:, o, :], y[:, o, :])
                                nc.tensor.matmul(
                                    ms_ps[0:1, :], ones[:], ysq[:],
                                    start=(o == 0), stop=(o == DK - 1))
                            mu = ep.tile([1, QT], f32, tag="mu", name=f"mu{g}_{qt}")
                            nc.scalar.mul(mu[:], mu_ps[0:1, :], 1.0 / D)
                            ms = ep.tile([1, QT], f32, tag="ms", name=f"ms{g}_{qt}")
                            nc.scalar.mul(ms[:], ms_ps[0:1, :], 1.0 / D)
                            musq = ep.tile([1, QT], f32, tag="musq",
                                           name=f"musq{g}_{qt}")
                            nc.scalar.square(musq[:], mu[:])
                            var = ep.tile([1, QT], f32, tag="var", name=f"var{g}_{qt}")
                            nc.vector.tensor_sub(var[:], ms[:], musq[:])
                            lnv = ep.tile([1, QT], f32, tag="lnv", name=f"lnv{g}_{qt}")
                            nc.scalar.activation(lnv[:], var[:], AF.Ln, bias=eps[:])
                            rstd = ep.tile([1, QT], f32, tag="rstd",
                                           name=f"rstd{g}_{qt}")
                            nc.scalar.activation(rstd[:], lnv[:], AF.Exp, scale=-0.5)
                            mu_bc = ep.tile([P, QT], f32, tag="mu_bc", bufs=1,
                                            name=f"mubc{g}_{qt}")
                            nc.gpsimd.partition_broadcast(mu_bc[:], mu[:])
                            rs_bc = ep.tile([P, QT], f32, tag="rs_bc", bufs=1,
                                            name=f"rsbc{g}_{qt}")
                            nc.gpsimd.partition_broadcast(rs_bc[:], rstd[:])
                            for o in range(DK):
                                t1 = ep.tile([P, QT], f32, tag="t1",
                                             name=f"t1{g}_{qt}_{o}")
                                nc.vector.tensor_sub(
                                    t1[:], y[:, o, :].bitcast(f32), mu_bc[:])
                                t2 = ep.tile([P, QT], f32, tag="t2",
                                             name=f"t2{g}_{qt}_{o}")
                                nc.vector.scalar_tensor_tensor(
                                    t2[:], t1[:], gam_(o), rs_bc[:],
                                    AluOpType.mult, AluOpType.mult,
                                )
                                yo = ep.tile([P, QT], f32, tag="yo",
                                             name=f"yo{g}_{qt}_{o}")
                                nc.vector.tensor_scalar_add(yo[:], t2[:], bet_(o))
                                nc.sync.dma_start(yt[ts(o, P), ts(qt, QT)], yo[:])

            pid = nc.partition_id()
            with tc.If(pid < 4) as cmp:
                body(0)
            with cmp.Else():
                body(1)
    nc.compile()
    return nc


def _get_nc():
    if "nc" not in _cache:
        _cache["nc"] = _build()
    return _cache["nc"]


def _prep(x, in_proj_w, in_proj_b, out_w, out_b, gamma, beta):
    import ml_dtypes
    bf16 = ml_dtypes.bfloat16

    x = np.asarray(x, np.float32)
    w = np.asarray(in_proj_w, np.float32)
    wt = np.ascontiguousarray(w.T)                          # [D, 3D]
    wo = np.asarray(out_w, np.float32)
    wot = np.ascontiguousarray(wo.T)                        # [D, D]
    bqkv = np.asarray(in_proj_b, np.float32)
    bo2 = np.asarray(out_b, np.float32) + wo @ bqkv[2 * D:3 * D]
    gam = np.asarray(gamma, np.float32)
    bet = np.asarray(beta, np.float32)
    tri = (np.arange(P)[:, None] <= np.arange(P)[None, :])
    mskd = np.ascontiguousarray(
        np.concatenate([tri, tri], axis=1).astype(bf16))
    wqkv_b = wt.astype(bf16)
    wot_b = wot.astype(bf16)
    qcols = {
        0: np.r_[0:QT, 3 * QT:4 * QT],
        1: np.r_[QT:3 * QT],
    }
    in_maps = []
    for c in range(8):
        b, g = c % 4, c // 4
        xt32 = np.ascontiguousarray(x[b].T)
        in_maps.append({
            "xtb": xt32.astype(bf16),
            "xq32": np.ascontiguousarray(xt32[:, qcols[g]]),
            "wqkv": wqkv_b,
            "wotd": wot_b,
            "mskd": mskd,
            "bqd": bqkv[0:D], "bkd": bqkv[D:2 * D], "bod": bo2,
            "gamd": gam, "betd": bet,
        })
    return in_maps, qcols


def _run(in_maps, trace=False, **kw):
    from concourse.bass_utils import run_bass_kernel_spmd

    return run_bass_kernel_spmd(_get_nc(), in_maps, list(range(8)), trace=trace, **kw)


def kernel(x, in_proj_w, in_proj_b, out_w, out_b, gamma, beta):
    in_maps, qcols = _prep(x, in_proj_w, in_proj_b, out_w, out_b, gamma, beta)
    res = _run(in_maps)
    out = np.empty((B, S, D), np.float32)
    for c in range(8):
        out[c % 4, qcols[c // 4]] = res.results[c]["yt"].T
    return out
